# revision 37
# baseline (speedup 1.0000x reference)
"""MLA QKV projection kernel for Trainium2 (8 NeuronCores, Bass/Tile).

Computes the DeepSeek-MLA q/k/v projection:
  q  = rmsnorm(hs @ q_a_w.T) @ q_b_w.T          -> [b, H, s, 192]  (pe cols rope-interleaved)
  ckv = hs @ kv_a_w.T ; compressed, k_pe = split
  kv = rmsnorm(compressed) @ kv_b_w.T           -> k_nope, v
  out = concat([q, concat(k_nope, k_pe), pad(v)], head axis)  -> [b, 3H, s, 192]

Strategy: pure data-parallel over tokens (8192 tokens -> 1024/core); all
weights replicated, every weight byte fetched from HBM exactly once per
core. All tensors are bf16 on the wire and in the matmuls (PSUM accum is
f32), which halves both DMA traffic and SBUF footprint vs f32/f32r. The
RMSNorm weights are folded into the B projection weights on the host; the
per-token rsqrt scale is applied to the B-GEMM outputs at PSUM-eviction
time (the scale commutes with the GEMM). The rope interleave permutation
is folded into weight rows on the host. Weights are pre-tiled on the host
into [chunk][k][128][512] blobs so every weight DMA is one fully
contiguous 128KB block; outputs are dumped as raw [chunk][m][128][512]
tiles (contiguous 128KB DMAs) and reassembled on the host.
"""

import sys
import types

import numpy as np

# ---- constants (hardcoded problem shape) ----
H = 32
D_NOPE = 128
D_ROPE = 64
D_Q = 192
D_V = 128
R_KV = 512
RQ = 1536
DMODEL = 4096
EPS = 1e-6
B, S = 2, 4096
NTOK = B * S            # 8192
NCORES = 8
TPC = NTOK // NCORES    # 1024 tokens per core
MT = TPC // 128         # 8 m-tiles
KT = DMODEL // 128      # 32 k-tiles for the A GEMM

A_COLS = RQ + R_KV + D_ROPE   # 2112 ( q 0:1536 | kv 1536:2048 | kpe 2048:2112 )
QOUT = H * D_Q                # 6144
KVOUT = H * (D_NOPE + D_V)    # 8192
QCH = QOUT // 512             # 12 B-q chunks
KVCH = KVOUT // 512           # 16 B-kv chunks
QKT = RQ // 128               # 12
KVKT = R_KV // 128            # 4
NSL = QKT + KVKT              # 16 transposed a slices


def _ensure_env():
    for p in ("/opt/trn_rl_repo", "/root/.axon_site"):
        if p not in sys.path:
            sys.path.insert(0, p)
    # bass_utils under axon wants antenv.axon_hooks; provide a stub so
    # trace=False runs never trip on the missing module.
    if "antenv.axon_hooks" not in sys.modules:
        try:
            import antenv  # noqa: F401
            import antenv.axon_hooks  # noqa: F401
        except ImportError:
            mod = types.ModuleType("antenv.axon_hooks")
            mod._hook = None
            mod.set_axon_ntff_profile_hook = lambda h: setattr(mod, "_hook", h)
            mod.get_axon_ntff_profile_hook = lambda: mod._hook
            sys.modules["antenv.axon_hooks"] = mod
            try:
                import antenv
                antenv.axon_hooks = mod
            except ImportError:
                pass


def _perm64():
    # inverse view of x.reshape(32,2).swapaxes ->  y[k] = x[2*(k%32) + k//32]
    return np.array([2 * (k % 32) + k // 32 for k in range(64)], dtype=np.int64)


_CACHE = {}


def _build():
    if "nc" in _CACHE:
        return _CACHE["nc"]
    _ensure_env()
    from concourse import bacc
    import concourse.mybir as mybir
    import concourse.tile as tile
    from concourse.masks import make_identity

    F32 = mybir.dt.float32
    BF16 = mybir.dt.bfloat16
    AF = mybir.ActivationFunctionType
    ALU = mybir.AluOpType
    AX = mybir.AxisListType

    nc = bacc.Bacc("TRN2", target_bir_lowering=False, debug=False)
    hsT_d = nc.dram_tensor("hsT", [DMODEL, TPC], BF16, kind="ExternalInput")
    wa4_d = nc.dram_tensor("wa4", [4, KT, 128, 512], BF16, kind="ExternalInput")
    wpe_d = nc.dram_tensor("wpe", [KT, 128, D_ROPE], BF16, kind="ExternalInput")
    qb_d = nc.dram_tensor("qb", [QCH, QKT, 128, 512], BF16, kind="ExternalInput")
    kvb_d = nc.dram_tensor("kvb", [KVCH, KVKT, 128, 512], BF16, kind="ExternalInput")
    outq_d = nc.dram_tensor("outq", [QCH, MT, 128, 512], BF16, kind="ExternalOutput")
    outkv_d = nc.dram_tensor("outkv", [KVCH, MT, 128, 512], BF16, kind="ExternalOutput")
    outpe_d = nc.dram_tensor("outpe", [D_ROPE, TPC], BF16, kind="ExternalOutput")

    with tile.TileContext(nc) as tc:
        with tc.tile_pool(name="persist", bufs=1) as persist:
            hst = persist.tile([128, KT, TPC], BF16)        # 64KB/p
            a_sb = persist.tile([128, MT, 2048], BF16)      # 32KB/p (q|kv cols)
            at = persist.tile([128, NSL, TPC], BF16)        # 32KB/p transposed a
            stats = persist.tile([128, MT, 4], F32)         # per-chunk sumsq
            s_q = persist.tile([128, MT], F32)
            s_kv = persist.tile([128, MT], F32)
            ident = persist.tile([128, 128], BF16)
            make_identity(nc, ident)

            for k in range(KT):
                nc.scalar.dma_start(out=hst[:, k, :],
                                    in_=hsT_d[k * 128:(k + 1) * 128, :])

            # ---------------- phase A: fused A GEMM ----------------
            # chunks: 4x512 (q,q,q,kv). Loop m innermost so each weight tile
            # is fetched once; all 8 PSUM banks accumulate.
            with tc.tile_pool(name="wa", bufs=8) as wap, \
                 tc.tile_pool(name="sqs", bufs=1) as sqsp:
                with tc.tile_pool(name="psA", bufs=1, space="PSUM") as psAp:
                    psA = psAp.tile([128, MT, 512], F32)        # all 8 banks
                    scr = sqsp.tile([128, 512], F32)
                    scr2 = sqsp.tile([128, 512], F32)
                    for c in range(4):
                        for k in range(KT):
                            wa_t = wap.tile([128, 512], BF16, tag="wa_t")
                            nc.sync.dma_start(out=wa_t, in_=wa4_d[c, k])
                            for m in range(MT):
                                nc.tensor.matmul(psA[:, m, :],
                                                 hst[:, k, m * 128:(m + 1) * 128],
                                                 wa_t,
                                                 start=(k == 0), stop=(k == KT - 1))
                        # evict psum via copies only (2-engine split) so banks
                        # free fast; sumsq reads the bf16 copy afterwards
                        for m in range(MT):
                            asl = a_sb[:, m, c * 512:(c + 1) * 512]
                            if m % 2 == 0:
                                nc.vector.tensor_copy(asl, psA[:, m, :512])
                            else:
                                nc.scalar.activation(asl, psA[:, m, :512], AF.Copy)
                        for m in range(MT):
                            asl = a_sb[:, m, c * 512:(c + 1) * 512]
                            if m % 2 == 0:
                                nc.scalar.activation(scr, asl, AF.Square,
                                                     accum_out=stats[:, m, c:c + 1])
                            else:
                                nc.vector.scalar_tensor_tensor(
                                    out=scr2, in0=asl, scalar=1.0, in1=asl,
                                    op0=ALU.mult, op1=ALU.mult,
                                    accum_out=stats[:, m, c:c + 1])


                # per-token rsqrt scales (all m at once)
                with tc.tile_pool(name="scl", bufs=1) as sclp:
                    for (dst, c0, c1, dim) in ((s_q, 0, 3, RQ), (s_kv, 3, 4, R_KV)):
                        t = sclp.tile([128, MT], F32)
                        nc.vector.reduce_sum(out=t, in_=stats[:, :, c0:c1], axis=AX.X)
                        nc.vector.tensor_scalar(out=t, in0=t, scalar1=1.0 / dim,
                                                scalar2=EPS, op0=ALU.mult, op1=ALU.add)
                        nc.vector.reciprocal(t, t)
                        nc.scalar.activation(dst[:, :], t, AF.Sqrt)

                # ---- phase T: transpose a -> at on the tensor engine, then
                # the kpe mini-GEMM, computed transposed ([64 pe-cols, tok];
                # stationary = wpe tile, moving = hst) so its matmuls run at
                # full rate with two bank-aligned PSUM accumulation groups,
                # then flipped back token-major via the DMA XBAR transpose.
                # The a-transpose psum->sbuf copies overlap the kpe matmuls.
                with tc.tile_pool(name="psT", bufs=4, space="PSUM") as psTp, \
                     tc.tile_pool(name="kpet", bufs=1) as kpetp, \
                     tc.tile_pool(name="psK", bufs=1, space="PSUM") as psKp:
                    # scalar is ~1.5x faster per psum->sbuf copy than vector
                    # here, so it takes 10 of the 16; vector takes 6 + the
                    # kpet eviction. Both chains hide under the kpe matmuls.
                    for i, (m, fb) in enumerate((m, fb) for m in range(MT)
                                                for fb in range(2)):
                        pt = psTp.tile([128, 8, 128], BF16, tag="pt")
                        for fi in range(8):
                            f = fb * 8 + fi
                            nc.tensor.transpose(pt[:, fi, :],
                                                a_sb[:, m, f * 128:(f + 1) * 128],
                                                ident)
                        dst = at[:, fb * 8:(fb + 1) * 8, m * 128:(m + 1) * 128]
                        if i in (0, 3, 6, 9, 12):
                            nc.vector.tensor_copy(dst, pt)
                        else:
                            nc.scalar.activation(dst, pt, AF.Copy)
                    psK = psKp.tile([D_ROPE, 2, 512], F32)      # 2 banks
                    kpet = kpetp.tile([D_ROPE, 2, 512], BF16)
                    for k in range(KT):
                        wa_t = wap.tile([128, 512], BF16, tag="wa_t")
                        nc.sync.dma_start(out=wa_t[:, :D_ROPE], in_=wpe_d[k])
                        for h in range(2):
                            nc.tensor.matmul(psK[:, h, :],
                                             wa_t[:, :D_ROPE],
                                             hst[:, k, h * 512:(h + 1) * 512],
                                             start=(k == 0), stop=(k == KT - 1))
                    nc.vector.tensor_copy(kpet, psK)
                    # kpe leaves the core column-major; the host flips it
                    nc.gpsimd.dma_start(out=outpe_d.rearrange("p (h w) -> p h w", h=2),
                                        in_=kpet)

            # ---------------- phase B: B GEMMs ----------------
            with tc.tile_pool(name="wb", bufs=12) as wbp, \
                 tc.tile_pool(name="ev", bufs=12) as evp, \
                 tc.tile_pool(name="psB", bufs=1, space="PSUM") as psBp:
                psB = psBp.tile([128, MT, 512], F32)        # all 8 banks

                def bchunk(c, nkt, ksl0, w_d, out_d, s_t, last=False):
                    for k in range(nkt):
                        wb_t = wbp.tile([128, 512], BF16, tag="wb_t")
                        nc.sync.dma_start(out=wb_t, in_=w_d[c, k])
                        for m in range(MT):
                            nc.tensor.matmul(psB[:, m, :],
                                             at[:, ksl0 + k, m * 128:(m + 1) * 128],
                                             wb_t,
                                             start=(k == 0), stop=(k == nkt - 1))
                    for m in range(MT):
                        ev = evp.tile([128, 512], BF16, tag="ev")
                        if m % 2 == 0:
                            nc.scalar.activation(ev, psB[:, m, :], AF.Copy,
                                                 scale=s_t[:, m:m + 1])
                        else:
                            nc.vector.tensor_scalar(out=ev, in0=psB[:, m, :],
                                                    scalar1=s_t[:, m:m + 1],
                                                    scalar2=None, op0=ALU.mult)
                        if last:
                            # the gpsimd queue is clogged with epilogue work
                            # at kernel end; the hwdge queues are idle by now
                            trig = nc.sync if m % 2 == 0 else nc.scalar
                        else:
                            trig = nc.gpsimd
                        trig.dma_start(out=out_d[c, m], in_=ev)

                for c in range(QCH):
                    bchunk(c, QKT, 0, qb_d, outq_d, s_q)
                for c in range(KVCH):
                    bchunk(c, KVKT, QKT, kvb_d, outkv_d, s_kv,
                           last=(c >= KVCH - 2))

    nc.compile()
    _CACHE["nc"] = nc
    return nc


def _prep_inputs(hidden_states, q_a_w, kv_a_w, q_b_w, kv_b_w, q_a_ln_w, kv_a_ln_w):
    import ml_dtypes
    f32 = np.float32
    bf16 = ml_dtypes.bfloat16
    perm = _perm64()

    hs = np.asarray(hidden_states, dtype=f32).reshape(NTOK, DMODEL).astype(bf16)
    hsT = hs.T                                             # [4096, 8192] view

    q_a_w = np.asarray(q_a_w, dtype=f32)
    kv_a_w = np.asarray(kv_a_w, dtype=f32)
    kv_a_pe = kv_a_w[R_KV:][perm]                          # de-interleave k_pe rows
    wa = np.concatenate([q_a_w, kv_a_w[:R_KV], kv_a_pe], axis=0)   # [2112, 4096]
    W = np.ascontiguousarray(wa.T).reshape(KT, 128, A_COLS)
    wa4 = np.ascontiguousarray(
        np.stack([W[:, :, c * 512:(c + 1) * 512] for c in range(4)])).astype(bf16)
    wpe = np.ascontiguousarray(W[:, :, 2048:2112]).astype(bf16)

    qb = np.asarray(q_b_w, dtype=f32) * np.asarray(q_a_ln_w, dtype=f32)[None, :]
    qb = qb.reshape(H, D_Q, RQ).copy()
    qb[:, D_NOPE:, :] = qb[:, D_NOPE + perm, :]            # de-interleave q_pe rows
    Q = np.ascontiguousarray(qb.reshape(QOUT, RQ).T).reshape(QKT, 128, QOUT)
    qbt = np.ascontiguousarray(
        np.stack([Q[:, :, c * 512:(c + 1) * 512] for c in range(QCH)])).astype(bf16)

    kvb = np.asarray(kv_b_w, dtype=f32) * np.asarray(kv_a_ln_w, dtype=f32)[None, :]
    KV = np.ascontiguousarray(kvb.T).reshape(KVKT, 128, KVOUT)
    kvbt = np.ascontiguousarray(
        np.stack([KV[:, :, c * 512:(c + 1) * 512] for c in range(KVCH)])).astype(bf16)

    in_maps = []
    for c in range(NCORES):
        in_maps.append({
            "hsT": np.ascontiguousarray(hsT[:, c * TPC:(c + 1) * TPC]),
            "wa4": wa4,
            "wpe": wpe,
            "qb": qbt,
            "kvb": kvbt,
        })
    return in_maps


def kernel(hidden_states, q_a_w, q_b_w, kv_a_w, kv_b_w, q_a_ln_w, kv_a_ln_w,
           _trace=False):
    _ensure_env()
    from concourse.bass_utils import run_bass_kernel_spmd

    nc = _build()
    in_maps = _prep_inputs(hidden_states, q_a_w, kv_a_w, q_b_w, kv_b_w,
                           q_a_ln_w, kv_a_ln_w)
    res = run_bass_kernel_spmd(nc, in_maps, list(range(NCORES)), trace=_trace)

    f32 = np.float32
    out = np.empty((B, 3 * H, S, D_Q), dtype=f32)
    for c in range(NCORES):
        bb, t0 = c // (S // TPC), (c % (S // TPC)) * TPC
        r = res.results[c]
        # q: [12 chunks, 8 m, 128, 512] -> [tok, 6144] -> [32, tok, 192]
        qf = r["outq"].transpose(1, 2, 0, 3).reshape(TPC, QOUT).astype(f32)
        out[bb, :H, t0:t0 + TPC, :] = qf.reshape(TPC, H, D_Q).transpose(1, 0, 2)
        # kv: [16, 8, 128, 512] -> [tok, 8192] -> [32, tok, 256]
        kvf = r["outkv"].transpose(1, 2, 0, 3).reshape(TPC, KVOUT).astype(f32)
        kvf = kvf.reshape(TPC, H, D_NOPE + D_V).transpose(1, 0, 2)
        out[bb, H:2 * H, t0:t0 + TPC, :D_NOPE] = kvf[:, :, :D_NOPE]
        out[bb, 2 * H:, t0:t0 + TPC, :D_V] = kvf[:, :, D_NOPE:]
        # k_pe broadcast to all key heads (device leaves it column-major)
        pe = r["outpe"].astype(f32).T
        out[bb, H:2 * H, t0:t0 + TPC, D_NOPE:] = pe[None, :, :]
    out[:, 2 * H:, :, D_V:] = 0.0      # v padding is exact zeros
    if _trace:
        kernel.last_exec_time_ns = res.exec_time_ns
        kernel.last_results = res
    return out
